# revision 38
# baseline (speedup 1.0000x reference)
"""Sparse dual-masked attention for Trainium2, 8 NeuronCores.

Problem: B=2, N=2048, DIM=512, H=8, DH=64.
  qkv = x @ W_qkv; per-head attention with dual mask
  (np_i*np_j==0 | bert_j==1 -> -1000), softmax, out proj + bias.

Key structure exploited (sparse_attention):
  - A row i with np_i==0 is fully masked -> softmax is uniform -> output row
    is the constant mean(V) @ W_out + b_out (computed on host; tiny).
  - For rows with np_i==1, only columns with np_j==1 & bert_j==0 survive
    (exp(-1000-max) == 0 exactly in the reference). So we gather those
    ~R=1030 rows and ~M=535 columns on the host and run a dense attention
    over the gathered set on device: ~8x less work than dense.

Sharding: core = (batch b, head-pair g): 2 batches x 4 head groups.
  W_qkv is split column-wise per head pair, W_out row-wise; each core
  produces a partial [R,512] output; host sums the 4 partials per batch.

v2: all matmul operands in bf16 (1 cyc/row on the PE vs 4 for fp32;
measured end-to-end rounding error ~3e-3 scale-relative, well under the
2e-2 gate). exp batched over multi-bank PSUM tiles to cut ScalarE
instruction overhead; S->exp->O software-pipelined by one (h,chunk)
iteration so the PE never waits on ScalarE; output projection DMA'd
straight from PSUM (no SBUF staging copies); denominator reciprocal
replicated across partitions by a gpsimd partition_broadcast instead of
a rank-1 matmul (frees a PSUM bank and the PE).

Device dataflow per core (R=R_PAD rows, M=M_PAD kv cols, 2 heads):
  x shipped pre-transposed/gathered as xT [512, R] (kv rows first, then
  tail rows); kvc is the kv-indicator column per m-tile, written into
  V's ones-columns so attn @ V also yields the softmax denominators.
  1. K^T = Wk^T x^T; Q^T = (0.125*Wq)^T x^T; V_aug per m-tile.
  2. per (h, r-chunk): S^T = K_h Q_h^T into grouped PSUM banks,
     P^T = exp(S^T) (ScalarE, one op per bank group, bf16 out).
  3. O^T[h] = V_aug_h^T P^T accumulated over m-tiles; row 0 is the
     denominator; recip (DVE) -> partition_broadcast (Pool) ->
     tensor_mul normalize into OnT (bf16).
  4. y = OnT^T Wo per 128-row tile, DMA'd from PSUM to HBM in f32
     (host sums the 4 partials per batch and adds the bias).
"""

import numpy as np

_CORES = 8
_DIM = 512
_DH = 64
_H = 8
_INNER = _H * _DH

# fallbacks (flip if a feature fails on sim/hw)
_PBCAST = True     # gpsimd partition_broadcast for recip replication
_PSUM_DMA = True   # DMA y straight from PSUM (no SBUF staging)


def _ceil_to(x, m):
    return ((x + m - 1) // m) * m


def build_bass(R_PAD, M_PAD):
    """Build the SPMD bass program for padded sizes R_PAD (queries) and
    M_PAD (kv columns). Returns the compiled Bacc object."""
    import concourse.bacc as bacc
    import concourse.mybir as mybir
    import concourse.tile as tile

    f32 = mybir.dt.float32
    f32r = mybir.dt.float32r
    bf16 = mybir.dt.bfloat16
    EXP = mybir.ActivationFunctionType.Exp

    assert R_PAD % 16 == 0 and M_PAD % 128 == 0 and R_PAD >= M_PAD
    NMT = M_PAD // 128          # kv m-tiles
    NRT = (R_PAD + 127) // 128  # query r-tiles for the final projection
    # r-chunks: big chunks first, then progressively smaller ones so the
    # trailing normalize->project->copy->DMA chain after the last chunk
    # covers as few output tiles as possible. Chunks stay 128-aligned so
    # each output r-tile belongs to exactly one chunk.
    RC = []
    o = 0
    while o < R_PAD:
        left = R_PAD - o
        if left > 512 + 384:
            w = 512
        elif left > 512:
            w = left - 128 - (left - 128) % 128 if left - 128 > 128 else 128
            w = min(w, 384)
        elif left > 128:
            w = left - (16 if left % 128 else 128)
            w -= w % 128
            w = max(w, 128)
        else:
            w = left
        RC.append((o, w))
        o += w
    MC = []
    o = 0
    while o < M_PAD:
        MC.append((o, min(512, M_PAD - o)))
        o += 512

    nc = bacc.Bacc("TRN2", target_bir_lowering=False, debug=False,
                   num_devices=_CORES)

    xT_d = nc.dram_tensor("xT", [512, R_PAD], bf16, kind="ExternalInput")
    wq_d = nc.dram_tensor("wq", [512, 128], bf16, kind="ExternalInput")
    wk_d = nc.dram_tensor("wk", [512, 128], bf16, kind="ExternalInput")
    wv_d = nc.dram_tensor("wv", [512, 128], bf16, kind="ExternalInput")
    kvc_d = nc.dram_tensor("kvc", [128, 2 * NMT], f32, kind="ExternalInput")
    wo_d = nc.dram_tensor("wo", [128, 512], bf16, kind="ExternalInput")
    y_d = nc.dram_tensor("y", [R_PAD, 512], bf16, kind="ExternalOutput")

    with tile.TileContext(nc) as tc:
        with (
            tc.tile_pool(name="consts", bufs=1) as consts,
            tc.tile_pool(name="pt", bufs=6) as ptpool,
            tc.tile_pool(name="rcp", bufs=3) as rpool,
            tc.tile_pool(name="ysb", bufs=3) as ypool,
            tc.tile_pool(name="psS0", bufs=1, space="PSUM") as psS0,
            tc.tile_pool(name="psS1", bufs=1, space="PSUM") as psS1,
            tc.tile_pool(name="po", bufs=2, space="PSUM") as po,
            tc.tile_pool(name="py", bufs=2, space="PSUM") as py,
        ):
            # ---- input DMAs: split across the two HWDGE queues (sync,
            # scalar) in need-order: the K projection consumes wk + xT
            # chunk-by-chunk first, so those transfers go to the front of
            # both queues (HBM read bandwidth is the serializer).
            wk = consts.tile([128, 4, 128], bf16, tag="wk")
            nc.sync.dma_start(
                out=wk, in_=wk_d.ap().rearrange("(a p) d -> p a d", p=128))
            wq = consts.tile([128, 4, 128], bf16, tag="wq")
            nc.scalar.dma_start(
                out=wq, in_=wq_d.ap().rearrange("(a p) d -> p a d", p=128))
            # xT streamed in half-chunks, alternating queues, so the K/Q
            # projection matmuls can start consuming chunk c while later
            # chunks are still in flight (the input phase is HBM-bound).
            xT = consts.tile([128, 4, R_PAD], bf16, tag="xT")
            HB = R_PAD // 2
            xeng = [nc.sync, nc.scalar]
            for c in range(4):
                for hf in range(2):
                    sl = slice(hf * HB, R_PAD if hf else HB)
                    xeng[(2 * c + hf) % 2].dma_start(
                        out=xT[:, c, sl],
                        in_=xT_d.ap()[c * 128:(c + 1) * 128, sl])
            wv = consts.tile([128, 4, 128], bf16, tag="wv")
            nc.sync.dma_start(
                out=wv, in_=wv_d.ap().rearrange("(a p) d -> p a d", p=128))
            kvc = consts.tile([128, NMT, 2], f32, tag="kvc")
            nc.scalar.dma_start(out=kvc, in_=kvc_d.ap())
            wo = consts.tile([128, 512], bf16, tag="wo")
            nc.scalar.dma_start(out=wo, in_=wo_d.ap())

            def psum_tile(i, name):
                # rotate [128,512] f32 psum scratch across the po/py pools
                pool = (po, py)[i % 2]
                return pool.tile([128, 512], f32, tag="big", name=name)



            # ---- phase 1: projections ------------------------------------
            # K first (S matmuls need all of KT), then Q chunk by chunk.
            # NOTE: gpsimd (Pool) cannot access PSUM, so every PSUM->SBUF
            # mover must be the scalar (activation) or vector (DVE) engine.
            KT = consts.tile([128, M_PAD], bf16, tag="KT")
            for i, (o, w) in enumerate(MC):
                ps = psum_tile(i, f"kps{i}")
                for c in range(4):
                    nc.tensor.matmul(ps[:, :w], wk[:, c, :], xT[:, c, o:o + w],
                                     start=(c == 0), stop=(c == 3))
                nc.vector.tensor_copy(KT[:, o:o + w], ps[:, :w])

            QT = consts.tile([128, R_PAD], bf16, tag="QT")
            for i, (o, w) in enumerate(RC):
                ps = psum_tile(i, f"qps{i}")
                for c in range(4):
                    nc.tensor.matmul(ps[:, :w], wq[:, c, :], xT[:, c, o:o + w],
                                     start=(c == 0), stop=(c == 3))
                nc.scalar.copy(QT[:, o:o + w], ps[:, :w])

            # V_aug layout per (mt, head): [kv1 | 63 zeros | V(64)] so the
            # attn@V output carries the softmax denominator at partition 0
            # and O at partitions 64:128. Rows are scaled by the kv
            # indicator to null tail rows sitting below M_PAD.
            vt = consts.tile([128, NMT, 2, 128], bf16, tag="vt")
            nc.gpsimd.memset(vt, 0.0)
            for mt in range(NMT):
                ps = psum_tile(mt, f"vps{mt}")
                sl = slice(mt * 128, (mt + 1) * 128)
                for c in range(4):
                    nc.tensor.matmul(ps[:, :128], xT[:, c, sl], wv[:, c, :],
                                     start=(c == 0), stop=(c == 3))
                for hh in range(2):
                    nc.gpsimd.tensor_copy(vt[:, mt, hh, 0:1],
                                          kvc[:, mt, hh:hh + 1])
                nc.vector.tensor_scalar_mul(
                    vt[:, mt, :, 64:128],
                    in0=ps[:, 0:128].rearrange("p (h c) -> p h c", h=2),
                    scalar1=kvc[:, mt, 0:1])

            # ---- phases 2+3, software-pipelined by one iteration ---------
            # iteration = (h, r-chunk); S matmuls of iter i overlap the
            # exp of iter i on ScalarE and the O/normalize of iter i-1.
            OnT = consts.tile([128, R_PAD], bf16, tag="OnT")

            # mt groupings: pairs of m-tiles share one 2-bank psum tile
            # and one exp instruction.
            GRPS = [(0, 1), (2, 3), (4,)]
            SPOOL = [psS0, psS1]

            def emit_S_pair(o, w, grps):
                """S^T matmuls + exp for both heads over the given m-tile
                groups. The two heads' matmuls are emitted adjacently: with
                64-partition contractions they occupy disjoint PE row
                groups and stream concurrently. Returns {h: [moving AP per
                m-tile, in order]}. Singleton groups and the tail chunk
                pack both heads into one psum tile / one exp."""
                pts = {0: [], 1: []}
                if w * NMT > 512:
                    for gi, g in grps:
                        ng = len(g)
                        if ng == 2:
                            sps = [SPOOL[h].tile([128, 2, 512], f32,
                                                 tag="sp",
                                                 name=f"sp{h}_{o}_{gi}")
                                   for h in (0, 1)]
                            for j, mt in enumerate(g):
                                msl = slice(mt * 128, (mt + 1) * 128)
                                for h in (0, 1):
                                    hs = slice(h * 64, (h + 1) * 64)
                                    nc.tensor.matmul(
                                        sps[h][:, j, :w], KT[hs, msl],
                                        QT[hs, o:o + w],
                                        start=True, stop=True)
                            for h in (0, 1):
                                pt = ptpool.tile([128, 2, 512], bf16,
                                                 tag="pt",
                                                 name=f"pt{h}_{o}_{gi}")
                                nc.scalar.activation(out=pt[:, :2, :w],
                                                     in_=sps[h][:, :2, :w],
                                                     func=EXP)
                                pts[h] += [pt[:, j, :w] for j in range(2)]
                        else:
                            # one m-tile: heads share the tile and the exp
                            mt = g[0]
                            msl = slice(mt * 128, (mt + 1) * 128)
                            sp = psS0.tile([128, 2, 512], f32, tag="sp",
                                           name=f"spm_{o}_{gi}")
                            for h in (0, 1):
                                hs = slice(h * 64, (h + 1) * 64)
                                nc.tensor.matmul(sp[:, h, :w], KT[hs, msl],
                                                 QT[hs, o:o + w],
                                                 start=True, stop=True)
                            pt = ptpool.tile([128, 2, 512], bf16, tag="pt",
                                             name=f"ptm_{o}_{gi}")
                            nc.scalar.activation(out=pt[:, :2, :w],
                                                 in_=sp[:, :2, :w], func=EXP)
                            for h in (0, 1):
                                pts[h].append(pt[:, h, :w])
                else:
                    # tail chunk: per head, all NMT m-tiles packed into one
                    # psum bank (head = slot); one exp for everything
                    if not grps or grps[0][0] != 0:
                        return pts
                    sp = psS0.tile([128, 2, 512], f32, tag="sp",
                                   name=f"spt_{o}")
                    for mt in range(NMT):
                        msl = slice(mt * 128, (mt + 1) * 128)
                        for h in (0, 1):
                            hs = slice(h * 64, (h + 1) * 64)
                            nc.tensor.matmul(sp[:, h, mt * w:(mt + 1) * w],
                                             KT[hs, msl], QT[hs, o:o + w],
                                             start=True, stop=True)
                    pt = ptpool.tile([128, 2, NMT, 16], bf16, tag="ptt",
                                     name=f"ptt_{o}")
                    nc.scalar.activation(
                        out=pt,
                        in_=sp[:, :2, :NMT * w].rearrange(
                            "p h (m c) -> p h m c", m=NMT),
                        func=EXP)
                    for h in (0, 1):
                        pts[h] += [pt[:, h, mt, :] for mt in range(NMT)]
                return pts

            def emit_O(h, o, w, mvs):
                """attn@V + normalize for one (h, r-chunk)."""
                ops = po.tile([128, 512], f32, tag="big", name=f"ops{h}_{o}")
                for mt, mv in enumerate(mvs):
                    nc.tensor.matmul(ops[:, :w], vt[:, mt, h, :], mv,
                                     start=(mt == 0), stop=(mt == NMT - 1))
                rcp = rpool.tile([1, 512], f32, tag="rcp", name=f"rcp{h}_{o}")
                nc.vector.reciprocal_approx_fast(rcp[:, :w], ops[0:1, :w])
                rep = rpool.tile([64, 512], f32, tag="rep", name=f"rep{h}_{o}")
                if _PBCAST:
                    nc.gpsimd.partition_broadcast(rep[:, :w], rcp[:, :w])
                else:
                    prep = py.tile([128, 512], f32, tag="big",
                                   name=f"prep{h}_{o}")
                    ones = getattr(emit_O, "_ones", None)
                    nc.tensor.matmul(prep[:64, :w],
                                     ones[0:1, :].bitcast(f32r),
                                     rcp[:, :w].bitcast(f32r),
                                     start=True, stop=True)
                    nc.vector.tensor_copy(rep[:, :w], prep[:64, :w])
                nc.vector.tensor_mul(OnT[h * 64:(h + 1) * 64, o:o + w],
                                     ops[64:128, :w], rep[:, :w])

            if not _PBCAST:
                ones = consts.tile([1, 64], f32, tag="ones")
                nc.vector.memset(ones, 1.0)
                emit_O._ones = ones

            ydone = 0
            cover = 0  # rows of OnT complete for both heads
            pend = None
            ycop = [nc.scalar, nc.vector]

            def emit_y(rt):
                tw = min(128, R_PAD - rt * 128)
                ps = py.tile([128, 512], f32, tag="big", name=f"yps{rt}")
                rsl = slice(rt * 128, rt * 128 + tw)
                nc.tensor.matmul(ps[:tw, :], OnT[:, rsl], wo,
                                 start=True, stop=True)
                ysb = ypool.tile([128, 512], bf16, tag="y", name=f"ysb{rt}")
                eng = ycop[rt % 2]
                if eng is nc.scalar:
                    eng.copy(ysb[:tw, :], ps[:tw, :])
                else:
                    eng.tensor_copy(ysb[:tw, :], ps[:tw, :])
                nc.sync.dma_start(out=y_d.ap()[rsl, :], in_=ysb[:tw, :])
            # iteration = r-chunk; O+normalize of the previous chunk is
            # interleaved between this chunk's S groups so the PE never
            # waits on the exp of the group it just produced. Output
            # r-tiles of a chunk are projected one iteration after that
            # chunk's normalize (the recip->broadcast->mul chain is
            # ~2.3us; the PE must not arrive early).
            ready = []   # r-tiles whose chunk normalized >= 1 iter ago
            fresh = []   # r-tiles normalized during this iteration
            for i, (o, w) in enumerate(RC):
                gl = list(enumerate(GRPS))
                pts = emit_S_pair(o, w, gl[:1])
                if pend is not None:
                    emit_O(0, po_, pw, pend[0])
                pts2 = emit_S_pair(o, w, gl[1:2])
                for rt in ready:
                    emit_y(rt)
                ready = []
                if pend is not None:
                    emit_O(1, po_, pw, pend[1])
                    fresh = list(range(po_ // 128, (po_ + pw + 127) // 128))
                pts3 = emit_S_pair(o, w, gl[2:])
                for h in (0, 1):
                    pts[h] = pts[h] + pts2[h] + pts3[h]
                pend, po_, pw = pts, o, w
                ready, fresh = ready + fresh, []
            emit_O(0, po_, pw, pend[0])
            emit_O(1, po_, pw, pend[1])
            for rt in ready + list(range(po_ // 128, (po_ + pw + 127) // 128)):
                emit_y(rt)

    nc.compile()
    return nc


def _prep(x, mask_np, mask_bert, W_qkv, W_out):
    """Host-side gather/shard. Returns (in_maps, meta)."""
    import ml_dtypes
    bf16 = ml_dtypes.bfloat16

    B, N, DIM = x.shape
    assert (B, DIM) == (2, _DIM)
    x = np.ascontiguousarray(x, dtype=np.float32)
    W_qkv = np.ascontiguousarray(W_qkv, dtype=np.float32)
    W_out = np.ascontiguousarray(W_out, dtype=np.float32)

    kv_idx, tail_idx, Ms, tails = [], [], [], []
    for b in range(B):
        npb = mask_np[b].astype(bool)
        bb = mask_bert[b].astype(bool)
        kv = np.nonzero(npb & ~bb)[0]
        tl = np.nonzero(npb & bb)[0]
        kv_idx.append(kv)
        tail_idx.append(tl)
        Ms.append(len(kv))
        tails.append(len(tl))

    M_PAD = max(128, _ceil_to(max(Ms), 128))
    # rows are packed [kv | tail] with no gap: the tail rows that fall in
    # [M_b, M_PAD) act as key/value candidates but are nulled by the kvc
    # indicator (V rows scaled to 0, denominator column 0), so no zero gap
    # is needed and R_PAD shrinks to the real row count.
    R_PAD = max(128, _ceil_to(max(Ms[b] + tails[b] for b in range(B)), 16),
                M_PAD)

    NMT = M_PAD // 128
    xT_b, kvc_b, row_pos = [], [], []
    for b in range(B):
        xa = np.zeros((512, R_PAD), dtype=bf16)
        xa[:, :Ms[b]] = x[b][kv_idx[b]].T.astype(bf16)
        xa[:, Ms[b]:Ms[b] + tails[b]] = x[b][tail_idx[b]].T.astype(bf16)
        xT_b.append(xa)
        kvones = np.zeros(M_PAD, dtype=np.float32)
        kvones[:Ms[b]] = 1.0
        # [128, NMT, 2]: per m-tile kv indicator, duplicated per head slot
        kvt = np.repeat(kvones.reshape(NMT, 128).T[:, :, None], 2, axis=2)
        kvc_b.append(np.ascontiguousarray(kvt.reshape(128, 2 * NMT)))
        # output row p of the device result corresponds to token row_pos[p]
        pos = np.concatenate([kv_idx[b], tail_idx[b]])
        row_pos.append(pos)

    scale = np.float32(_DH ** -0.5)
    in_maps = []
    for c in range(_CORES):
        b, g = divmod(c, 4)
        qc = slice(128 * g, 128 * g + 128)
        kc = slice(_INNER + 128 * g, _INNER + 128 * g + 128)
        vc = slice(2 * _INNER + 128 * g, 2 * _INNER + 128 * g + 128)
        wq = np.ascontiguousarray((W_qkv[:, qc] * scale).astype(bf16))
        wk = np.ascontiguousarray(W_qkv[:, kc].astype(bf16))
        wv = np.ascontiguousarray(W_qkv[:, vc].astype(bf16))
        wo = np.ascontiguousarray(
            W_out[128 * g:128 * g + 128, :].astype(bf16))
        in_maps.append({"xT": xT_b[b], "wq": wq, "wk": wk, "wv": wv,
                        "wo": wo, "kvc": kvc_b[b]})

    meta = dict(M_PAD=M_PAD, R_PAD=R_PAD, Ms=Ms, tails=tails,
                kv_idx=kv_idx, tail_idx=tail_idx, row_pos=row_pos)
    return in_maps, meta


def _assemble(results, meta, x, mask_np, W_qkv, W_out, b_out):
    B, N, _ = x.shape
    out = np.empty((B, N, _DIM), dtype=np.float32)
    Wv_full = W_qkv[:, 2 * _INNER:].astype(np.float32)
    for b in range(B):
        # constant output for fully-masked rows: uniform attention = mean(V)
        meanv = (x[b].mean(axis=0, dtype=np.float32) @ Wv_full)
        yconst = meanv @ W_out.astype(np.float32) + b_out
        out[b, :, :] = yconst[None, :]
        Mb, tb = meta["Ms"][b], meta["tails"][b]
        if Mb == 0:
            # no unmasked kv columns: every row is fully masked -> uniform
            continue
        acc = None
        for g in range(4):
            yp = np.asarray(results[4 * b + g]["y"], dtype=np.float32)
            acc = yp.copy() if acc is None else acc + yp
        out[b, meta["row_pos"][b], :] = acc[:Mb + tb] + b_out
    return out


_CACHE = {}


def _get_bass(R_PAD, M_PAD):
    key = (R_PAD, M_PAD)
    if key not in _CACHE:
        _CACHE[key] = build_bass(R_PAD, M_PAD)
    return _CACHE[key]


def run_spmd(in_maps, meta, trace=False, tmpdir=None, trace_cores=None):
    from concourse.bass_utils import run_bass_kernel_spmd

    nc = _get_bass(meta["R_PAD"], meta["M_PAD"])
    return run_bass_kernel_spmd(
        nc, in_maps, core_ids=list(range(_CORES)), trace=trace, tmpdir=tmpdir,
        trace_cores=trace_cores)


def kernel(x, mask_np, mask_bert, W_qkv, W_out, b_out):
    x = np.asarray(x)
    mask_np = np.asarray(mask_np)
    mask_bert = np.asarray(mask_bert)
    W_qkv = np.asarray(W_qkv, dtype=np.float32)
    W_out = np.asarray(W_out, dtype=np.float32)
    b_out = np.asarray(b_out, dtype=np.float32)

    in_maps, meta = _prep(x, mask_np, mask_bert, W_qkv, W_out)
    res = run_spmd(in_maps, meta)
    return _assemble(res.results, meta, x, mask_np, W_qkv, W_out, b_out)


# revision 39
# speedup vs baseline: 1.1764x; 1.1764x over previous
"""Sparse dual-masked attention for Trainium2, 8 NeuronCores.

Problem: B=2, N=2048, DIM=512, H=8, DH=64.
  qkv = x @ W_qkv; per-head attention with dual mask
  (np_i*np_j==0 | bert_j==1 -> -1000), softmax, out proj + bias.

Key structure exploited (sparse_attention):
  - A row i with np_i==0 is fully masked -> softmax is uniform -> output row
    is the constant mean(V) @ W_out + b_out (computed on host; tiny).
  - For rows with np_i==1, only columns with np_j==1 & bert_j==0 survive
    (exp(-1000-max) == 0 exactly in the reference). So we gather those
    ~R=1030 rows and ~M=535 columns on the host and run a dense attention
    over the gathered set on device: ~8x less work than dense.

Sharding: core = (batch b, head-pair g): 2 batches x 4 head groups.
  W_qkv is split column-wise per head pair, W_out row-wise; each core
  produces a partial [R,512] output; host sums the 4 partials per batch.

v2: all matmul operands in bf16 (1 cyc/row on the PE vs 4 for fp32;
measured end-to-end rounding error ~3e-3 scale-relative, well under the
2e-2 gate). exp batched over multi-bank PSUM tiles to cut ScalarE
instruction overhead; S->exp->O software-pipelined by one (h,chunk)
iteration so the PE never waits on ScalarE; output projection DMA'd
straight from PSUM (no SBUF staging copies); denominator reciprocal
replicated across partitions by a gpsimd partition_broadcast instead of
a rank-1 matmul (frees a PSUM bank and the PE).

Device dataflow per core (R=R_PAD rows, M=M_PAD kv cols, 2 heads):
  x shipped pre-transposed/gathered as xT [512, R] (kv rows first, then
  tail rows); kvc is the kv-indicator column per m-tile, written into
  V's ones-columns so attn @ V also yields the softmax denominators.
  1. K^T = Wk^T x^T; Q^T = (0.125*Wq)^T x^T; V_aug per m-tile.
  2. per (h, r-chunk): S^T = K_h Q_h^T into grouped PSUM banks,
     P^T = exp(S^T) (ScalarE, one op per bank group, bf16 out).
  3. O^T[h] = V_aug_h^T P^T accumulated over m-tiles; row 0 is the
     denominator; recip (DVE) -> partition_broadcast (Pool) ->
     tensor_mul normalize into OnT (bf16).
  4. y = OnT^T Wo per 128-row tile, DMA'd from PSUM to HBM in f32
     (host sums the 4 partials per batch and adds the bias).
"""

import numpy as np

_CORES = 8
_DIM = 512
_DH = 64
_H = 8
_INNER = _H * _DH

# fallbacks (flip if a feature fails on sim/hw)
_PBCAST = True     # gpsimd partition_broadcast for recip replication
_PSUM_DMA = True   # DMA y straight from PSUM (no SBUF staging)


def _ceil_to(x, m):
    return ((x + m - 1) // m) * m


def build_bass(R_PAD, M_PAD):
    """Build the SPMD bass program for padded sizes R_PAD (queries) and
    M_PAD (kv columns). Returns the compiled Bacc object."""
    import concourse.bacc as bacc
    import concourse.mybir as mybir
    import concourse.tile as tile

    f32 = mybir.dt.float32
    f32r = mybir.dt.float32r
    bf16 = mybir.dt.bfloat16
    EXP = mybir.ActivationFunctionType.Exp

    assert R_PAD % 16 == 0 and M_PAD % 128 == 0 and R_PAD >= M_PAD
    NMT = M_PAD // 128          # kv m-tiles
    NRT = (R_PAD + 127) // 128  # query r-tiles for the final projection
    # r-chunks: full 512s plus a short tail (bf16 matmuls don't need >=256;
    # finer chunking was measured slower: per-chunk pipeline overhead
    # outweighs the shorter trailing chain)
    RC = []
    o = 0
    while o < R_PAD:
        RC.append((o, min(512, R_PAD - o)))
        o += 512
    MC = []
    o = 0
    while o < M_PAD:
        MC.append((o, min(512, M_PAD - o)))
        o += 512

    nc = bacc.Bacc("TRN2", target_bir_lowering=False, debug=False,
                   num_devices=_CORES)

    xT_d = nc.dram_tensor("xT", [512, R_PAD], bf16, kind="ExternalInput")
    wq_d = nc.dram_tensor("wq", [512, 128], bf16, kind="ExternalInput")
    wk_d = nc.dram_tensor("wk", [512, 128], bf16, kind="ExternalInput")
    wv_d = nc.dram_tensor("wv", [512, 128], bf16, kind="ExternalInput")
    kvc_d = nc.dram_tensor("kvc", [128, 2 * NMT], f32, kind="ExternalInput")
    wo_d = nc.dram_tensor("wo", [128, 512], bf16, kind="ExternalInput")
    y_d = nc.dram_tensor("y", [R_PAD, 512], bf16, kind="ExternalOutput")

    with tile.TileContext(nc) as tc:
        with (
            tc.tile_pool(name="consts", bufs=1) as consts,
            tc.tile_pool(name="pt", bufs=6) as ptpool,
            tc.tile_pool(name="rcp", bufs=3) as rpool,
            tc.tile_pool(name="ysb", bufs=3) as ypool,
            tc.tile_pool(name="psS0", bufs=1, space="PSUM") as psS0,
            tc.tile_pool(name="psS1", bufs=1, space="PSUM") as psS1,
            tc.tile_pool(name="po", bufs=2, space="PSUM") as po,
            tc.tile_pool(name="py", bufs=2, space="PSUM") as py,
        ):
            # ---- input DMAs: split across the two HWDGE queues (sync,
            # scalar) in need-order: the K projection consumes wk + xT
            # chunk-by-chunk first, so those transfers go to the front of
            # both queues (HBM read bandwidth is the serializer).
            wk = consts.tile([128, 4, 128], bf16, tag="wk")
            nc.sync.dma_start(
                out=wk, in_=wk_d.ap().rearrange("(a p) d -> p a d", p=128))
            wq = consts.tile([128, 4, 128], bf16, tag="wq")
            nc.scalar.dma_start(
                out=wq, in_=wq_d.ap().rearrange("(a p) d -> p a d", p=128))
            # xT streamed in half-chunks, alternating queues, so the K/Q
            # projection matmuls can start consuming chunk c while later
            # chunks are still in flight (the input phase is HBM-bound).
            xT = consts.tile([128, 4, R_PAD], bf16, tag="xT")
            HB = R_PAD // 2
            xeng = [nc.sync, nc.scalar]
            for c in range(4):
                for hf in range(2):
                    sl = slice(hf * HB, R_PAD if hf else HB)
                    xeng[(2 * c + hf) % 2].dma_start(
                        out=xT[:, c, sl],
                        in_=xT_d.ap()[c * 128:(c + 1) * 128, sl])
            wv = consts.tile([128, 4, 128], bf16, tag="wv")
            nc.sync.dma_start(
                out=wv, in_=wv_d.ap().rearrange("(a p) d -> p a d", p=128))
            kvc = consts.tile([128, NMT, 2], f32, tag="kvc")
            nc.scalar.dma_start(out=kvc, in_=kvc_d.ap())
            wo = consts.tile([128, 512], bf16, tag="wo")
            nc.scalar.dma_start(out=wo, in_=wo_d.ap())

            def psum_tile(i, name):
                # rotate [128,512] f32 psum scratch across the po/py pools
                pool = (po, py)[i % 2]
                return pool.tile([128, 512], f32, tag="big", name=name)



            # ---- phase 1: projections ------------------------------------
            # K first (S matmuls need all of KT), then Q chunk by chunk.
            # NOTE: gpsimd (Pool) cannot access PSUM, so every PSUM->SBUF
            # mover must be the scalar (activation) or vector (DVE) engine.
            KT = consts.tile([128, M_PAD], bf16, tag="KT")
            for i, (o, w) in enumerate(MC):
                ps = psum_tile(i, f"kps{i}")
                for c in range(4):
                    nc.tensor.matmul(ps[:, :w], wk[:, c, :], xT[:, c, o:o + w],
                                     start=(c == 0), stop=(c == 3))
                nc.vector.tensor_copy(KT[:, o:o + w], ps[:, :w])

            QT = consts.tile([128, R_PAD], bf16, tag="QT")
            for i, (o, w) in enumerate(RC):
                ps = psum_tile(i, f"qps{i}")
                for c in range(4):
                    nc.tensor.matmul(ps[:, :w], wq[:, c, :], xT[:, c, o:o + w],
                                     start=(c == 0), stop=(c == 3))
                nc.scalar.copy(QT[:, o:o + w], ps[:, :w])

            # V_aug layout per (mt, head): [kv1 | 63 zeros | V(64)] so the
            # attn@V output carries the softmax denominator at partition 0
            # and O at partitions 64:128. Rows are scaled by the kv
            # indicator to null tail rows sitting below M_PAD.
            vt = consts.tile([128, NMT, 2, 128], bf16, tag="vt")
            nc.gpsimd.memset(vt, 0.0)
            for mt in range(NMT):
                ps = psum_tile(mt, f"vps{mt}")
                sl = slice(mt * 128, (mt + 1) * 128)
                for c in range(4):
                    nc.tensor.matmul(ps[:, :128], xT[:, c, sl], wv[:, c, :],
                                     start=(c == 0), stop=(c == 3))
                for hh in range(2):
                    nc.gpsimd.tensor_copy(vt[:, mt, hh, 0:1],
                                          kvc[:, mt, hh:hh + 1])
                nc.vector.tensor_scalar_mul(
                    vt[:, mt, :, 64:128],
                    in0=ps[:, 0:128].rearrange("p (h c) -> p h c", h=2),
                    scalar1=kvc[:, mt, 0:1])

            # ---- phases 2+3, software-pipelined by one iteration ---------
            # iteration = (h, r-chunk); S matmuls of iter i overlap the
            # exp of iter i on ScalarE and the O/normalize of iter i-1.
            OnT = consts.tile([128, R_PAD], bf16, tag="OnT")

            # mt groupings: pairs of m-tiles share one 2-bank psum tile
            # and one exp instruction.
            GRPS = [(0, 1), (2, 3), (4,)]
            SPOOL = [psS0, psS1]

            def emit_S_pair(o, w, grps):
                """S^T matmuls + exp for both heads over the given m-tile
                groups. The two heads' matmuls are emitted adjacently: with
                64-partition contractions they occupy disjoint PE row
                groups and stream concurrently. Returns {h: [moving AP per
                m-tile, in order]}. Singleton groups and the tail chunk
                pack both heads into one psum tile / one exp."""
                pts = {0: [], 1: []}
                if w * NMT > 512:
                    for gi, g in grps:
                        ng = len(g)
                        if ng == 2:
                            sps = [SPOOL[h].tile([128, 2, 512], f32,
                                                 tag="sp",
                                                 name=f"sp{h}_{o}_{gi}")
                                   for h in (0, 1)]
                            for j, mt in enumerate(g):
                                msl = slice(mt * 128, (mt + 1) * 128)
                                for h in (0, 1):
                                    hs = slice(h * 64, (h + 1) * 64)
                                    nc.tensor.matmul(
                                        sps[h][:, j, :w], KT[hs, msl],
                                        QT[hs, o:o + w],
                                        start=True, stop=True)
                            for h in (0, 1):
                                pt = ptpool.tile([128, 2, 512], bf16,
                                                 tag="pt",
                                                 name=f"pt{h}_{o}_{gi}")
                                nc.scalar.activation(out=pt[:, :2, :w],
                                                     in_=sps[h][:, :2, :w],
                                                     func=EXP)
                                pts[h] += [pt[:, j, :w] for j in range(2)]
                        else:
                            # one m-tile: heads share the tile and the exp
                            mt = g[0]
                            msl = slice(mt * 128, (mt + 1) * 128)
                            sp = psS0.tile([128, 2, 512], f32, tag="sp",
                                           name=f"spm_{o}_{gi}")
                            for h in (0, 1):
                                hs = slice(h * 64, (h + 1) * 64)
                                nc.tensor.matmul(sp[:, h, :w], KT[hs, msl],
                                                 QT[hs, o:o + w],
                                                 start=True, stop=True)
                            pt = ptpool.tile([128, 2, 512], bf16, tag="pt",
                                             name=f"ptm_{o}_{gi}")
                            nc.scalar.activation(out=pt[:, :2, :w],
                                                 in_=sp[:, :2, :w], func=EXP)
                            for h in (0, 1):
                                pts[h].append(pt[:, h, :w])
                else:
                    # tail chunk: per head, all NMT m-tiles packed into one
                    # psum bank (head = slot); one exp for everything
                    if not grps or grps[0][0] != 0:
                        return pts
                    sp = psS0.tile([128, 2, 512], f32, tag="sp",
                                   name=f"spt_{o}")
                    for mt in range(NMT):
                        msl = slice(mt * 128, (mt + 1) * 128)
                        for h in (0, 1):
                            hs = slice(h * 64, (h + 1) * 64)
                            nc.tensor.matmul(sp[:, h, mt * w:(mt + 1) * w],
                                             KT[hs, msl], QT[hs, o:o + w],
                                             start=True, stop=True)
                    pt = ptpool.tile([128, 2, NMT, 16], bf16, tag="ptt",
                                     name=f"ptt_{o}")
                    nc.scalar.activation(
                        out=pt,
                        in_=sp[:, :2, :NMT * w].rearrange(
                            "p h (m c) -> p h m c", m=NMT),
                        func=EXP)
                    for h in (0, 1):
                        pts[h] += [pt[:, h, mt, :] for mt in range(NMT)]
                return pts

            def emit_O(h, o, w, mvs):
                """attn@V + normalize for one (h, r-chunk)."""
                ops = po.tile([128, 512], f32, tag="big", name=f"ops{h}_{o}")
                for mt, mv in enumerate(mvs):
                    nc.tensor.matmul(ops[:, :w], vt[:, mt, h, :], mv,
                                     start=(mt == 0), stop=(mt == NMT - 1))
                rcp = rpool.tile([1, 512], f32, tag="rcp", name=f"rcp{h}_{o}")
                nc.vector.reciprocal_approx_fast(rcp[:, :w], ops[0:1, :w])
                rep = rpool.tile([64, 512], f32, tag="rep", name=f"rep{h}_{o}")
                if _PBCAST:
                    nc.gpsimd.partition_broadcast(rep[:, :w], rcp[:, :w])
                else:
                    prep = py.tile([128, 512], f32, tag="big",
                                   name=f"prep{h}_{o}")
                    ones = getattr(emit_O, "_ones", None)
                    nc.tensor.matmul(prep[:64, :w],
                                     ones[0:1, :].bitcast(f32r),
                                     rcp[:, :w].bitcast(f32r),
                                     start=True, stop=True)
                    nc.vector.tensor_copy(rep[:, :w], prep[:64, :w])
                nc.vector.tensor_mul(OnT[h * 64:(h + 1) * 64, o:o + w],
                                     ops[64:128, :w], rep[:, :w])

            if not _PBCAST:
                ones = consts.tile([1, 64], f32, tag="ones")
                nc.vector.memset(ones, 1.0)
                emit_O._ones = ones

            ydone = 0
            cover = 0  # rows of OnT complete for both heads
            pend = None
            ycop = [nc.scalar, nc.vector]

            def emit_y(rt):
                tw = min(128, R_PAD - rt * 128)
                ps = py.tile([128, 512], f32, tag="big", name=f"yps{rt}")
                rsl = slice(rt * 128, rt * 128 + tw)
                nc.tensor.matmul(ps[:tw, :], OnT[:, rsl], wo,
                                 start=True, stop=True)
                ysb = ypool.tile([128, 512], bf16, tag="y", name=f"ysb{rt}")
                eng = ycop[rt % 2]
                if eng is nc.scalar:
                    eng.copy(ysb[:tw, :], ps[:tw, :])
                else:
                    eng.tensor_copy(ysb[:tw, :], ps[:tw, :])
                nc.sync.dma_start(out=y_d.ap()[rsl, :], in_=ysb[:tw, :])
            # iteration = r-chunk; O+normalize of the previous chunk is
            # interleaved between this chunk's S groups so the PE never
            # waits on the exp of the group it just produced. Output
            # r-tiles of a chunk are projected one iteration after that
            # chunk's normalize (the recip->broadcast->mul chain is
            # ~2.3us; the PE must not arrive early).
            ready = []   # r-tiles whose chunk normalized >= 1 iter ago
            fresh = []   # r-tiles normalized during this iteration
            for i, (o, w) in enumerate(RC):
                gl = list(enumerate(GRPS))
                pts = emit_S_pair(o, w, gl[:1])
                if pend is not None:
                    emit_O(0, po_, pw, pend[0])
                pts2 = emit_S_pair(o, w, gl[1:2])
                for rt in ready:
                    emit_y(rt)
                ready = []
                if pend is not None:
                    emit_O(1, po_, pw, pend[1])
                    fresh = list(range(po_ // 128, (po_ + pw + 127) // 128))
                pts3 = emit_S_pair(o, w, gl[2:])
                for h in (0, 1):
                    pts[h] = pts[h] + pts2[h] + pts3[h]
                pend, po_, pw = pts, o, w
                ready, fresh = ready + fresh, []
            emit_O(0, po_, pw, pend[0])
            emit_O(1, po_, pw, pend[1])
            for rt in ready + list(range(po_ // 128, (po_ + pw + 127) // 128)):
                emit_y(rt)

    nc.compile()
    return nc


def _prep(x, mask_np, mask_bert, W_qkv, W_out):
    """Host-side gather/shard. Returns (in_maps, meta)."""
    import ml_dtypes
    bf16 = ml_dtypes.bfloat16

    B, N, DIM = x.shape
    assert (B, DIM) == (2, _DIM)
    x = np.ascontiguousarray(x, dtype=np.float32)
    W_qkv = np.ascontiguousarray(W_qkv, dtype=np.float32)
    W_out = np.ascontiguousarray(W_out, dtype=np.float32)

    kv_idx, tail_idx, Ms, tails = [], [], [], []
    for b in range(B):
        npb = mask_np[b].astype(bool)
        bb = mask_bert[b].astype(bool)
        kv = np.nonzero(npb & ~bb)[0]
        tl = np.nonzero(npb & bb)[0]
        kv_idx.append(kv)
        tail_idx.append(tl)
        Ms.append(len(kv))
        tails.append(len(tl))

    M_PAD = max(128, _ceil_to(max(Ms), 128))
    # rows are packed [kv | tail] with no gap: the tail rows that fall in
    # [M_b, M_PAD) act as key/value candidates but are nulled by the kvc
    # indicator (V rows scaled to 0, denominator column 0), so no zero gap
    # is needed and R_PAD shrinks to the real row count.
    R_PAD = max(128, _ceil_to(max(Ms[b] + tails[b] for b in range(B)), 16),
                M_PAD)

    NMT = M_PAD // 128
    xT_b, kvc_b, row_pos = [], [], []
    for b in range(B):
        xa = np.zeros((512, R_PAD), dtype=bf16)
        xa[:, :Ms[b]] = x[b][kv_idx[b]].T.astype(bf16)
        xa[:, Ms[b]:Ms[b] + tails[b]] = x[b][tail_idx[b]].T.astype(bf16)
        xT_b.append(xa)
        kvones = np.zeros(M_PAD, dtype=np.float32)
        kvones[:Ms[b]] = 1.0
        # [128, NMT, 2]: per m-tile kv indicator, duplicated per head slot
        kvt = np.repeat(kvones.reshape(NMT, 128).T[:, :, None], 2, axis=2)
        kvc_b.append(np.ascontiguousarray(kvt.reshape(128, 2 * NMT)))
        # output row p of the device result corresponds to token row_pos[p]
        pos = np.concatenate([kv_idx[b], tail_idx[b]])
        row_pos.append(pos)

    scale = np.float32(_DH ** -0.5)
    in_maps = []
    for c in range(_CORES):
        b, g = divmod(c, 4)
        qc = slice(128 * g, 128 * g + 128)
        kc = slice(_INNER + 128 * g, _INNER + 128 * g + 128)
        vc = slice(2 * _INNER + 128 * g, 2 * _INNER + 128 * g + 128)
        wq = np.ascontiguousarray((W_qkv[:, qc] * scale).astype(bf16))
        wk = np.ascontiguousarray(W_qkv[:, kc].astype(bf16))
        wv = np.ascontiguousarray(W_qkv[:, vc].astype(bf16))
        wo = np.ascontiguousarray(
            W_out[128 * g:128 * g + 128, :].astype(bf16))
        in_maps.append({"xT": xT_b[b], "wq": wq, "wk": wk, "wv": wv,
                        "wo": wo, "kvc": kvc_b[b]})

    meta = dict(M_PAD=M_PAD, R_PAD=R_PAD, Ms=Ms, tails=tails,
                kv_idx=kv_idx, tail_idx=tail_idx, row_pos=row_pos)
    return in_maps, meta


def _assemble(results, meta, x, mask_np, W_qkv, W_out, b_out):
    B, N, _ = x.shape
    out = np.empty((B, N, _DIM), dtype=np.float32)
    Wv_full = W_qkv[:, 2 * _INNER:].astype(np.float32)
    for b in range(B):
        # constant output for fully-masked rows: uniform attention = mean(V)
        meanv = (x[b].mean(axis=0, dtype=np.float32) @ Wv_full)
        yconst = meanv @ W_out.astype(np.float32) + b_out
        out[b, :, :] = yconst[None, :]
        Mb, tb = meta["Ms"][b], meta["tails"][b]
        if Mb == 0:
            # no unmasked kv columns: every row is fully masked -> uniform
            continue
        acc = None
        for g in range(4):
            yp = np.asarray(results[4 * b + g]["y"], dtype=np.float32)
            acc = yp.copy() if acc is None else acc + yp
        out[b, meta["row_pos"][b], :] = acc[:Mb + tb] + b_out
    return out


_CACHE = {}


def _get_bass(R_PAD, M_PAD):
    key = (R_PAD, M_PAD)
    if key not in _CACHE:
        _CACHE[key] = build_bass(R_PAD, M_PAD)
    return _CACHE[key]


def run_spmd(in_maps, meta, trace=False, tmpdir=None, trace_cores=None):
    from concourse.bass_utils import run_bass_kernel_spmd

    nc = _get_bass(meta["R_PAD"], meta["M_PAD"])
    return run_bass_kernel_spmd(
        nc, in_maps, core_ids=list(range(_CORES)), trace=trace, tmpdir=tmpdir,
        trace_cores=trace_cores)


def kernel(x, mask_np, mask_bert, W_qkv, W_out, b_out):
    x = np.asarray(x)
    mask_np = np.asarray(mask_np)
    mask_bert = np.asarray(mask_bert)
    W_qkv = np.asarray(W_qkv, dtype=np.float32)
    W_out = np.asarray(W_out, dtype=np.float32)
    b_out = np.asarray(b_out, dtype=np.float32)

    in_maps, meta = _prep(x, mask_np, mask_bert, W_qkv, W_out)
    res = run_spmd(in_maps, meta)
    return _assemble(res.results, meta, x, mask_np, W_qkv, W_out, b_out)


# revision 41
# speedup vs baseline: 1.1849x; 1.0072x over previous
"""Sparse dual-masked attention for Trainium2, 8 NeuronCores.

Problem: B=2, N=2048, DIM=512, H=8, DH=64.
  qkv = x @ W_qkv; per-head attention with dual mask
  (np_i*np_j==0 | bert_j==1 -> -1000), softmax, out proj + bias.

Key structure exploited (sparse_attention):
  - A row i with np_i==0 is fully masked -> softmax is uniform -> output row
    is the constant mean(V) @ W_out + b_out (computed on host; tiny).
  - For rows with np_i==1, only columns with np_j==1 & bert_j==0 survive
    (exp(-1000-max) == 0 exactly in the reference). So we gather those
    ~R=1030 rows and ~M=535 columns on the host and run a dense attention
    over the gathered set on device: ~8x less work than dense.

Sharding: core = (batch b, head-pair g): 2 batches x 4 head groups.
  W_qkv is split column-wise per head pair, W_out row-wise; each core
  produces a partial [R,512] output; host sums the 4 partials per batch.

v2+ (2.2x vs the fp32 baseline): every matmul operand in bf16 (1 cyc/row
on the PE vs 4 for fp32; measured end-to-end rounding error 3.5e-3
scale-relative vs the 2e-2 gate). exp batched over 2-bank PSUM tiles
(one ScalarE op per 2 m-tiles; single-m-tile groups and the 16-row tail
merge both heads into one op). The r-chunk loop is software-pipelined:
chunk i's S matmuls + exps overlap chunk i-1's attn@V + normalize, and
the out-projection of a chunk trails its normalize by one full chunk so
the PE never waits on the DVE normalize chain. The softmax-denominator
reciprocal is replicated across partitions by a gpsimd
partition_broadcast (SBUF->SBUF; gpsimd cannot touch PSUM) instead of a
rank-1 matmul. Inputs stream in half-chunks in need-order across the
two HWDGE queues (sync/scalar) so the K/Q projections start while the
rest of x is still in flight. Measured non-wins kept out: PE warm-up
matmuls (trip the HW duty-cycle governor: the chip clamps the PE to
k=4/n=8 duty in 3.4us epochs when the power integral runs hot, so
wasted rows cost real time), finer trailing r-chunks (per-chunk
pipeline overhead exceeded the shorter drain), adjacent head-paired S
matmuls (the PE moving-data path is a single stream; 64-partition row
groups do not run concurrently).

Device dataflow per core (R=R_PAD rows, M=M_PAD kv cols, 2 heads):
  x shipped pre-transposed/gathered as xT [512, R] bf16 (kv rows first,
  then tail rows); kvc is the kv-indicator column per m-tile, written
  into V_aug's ones-columns so attn @ V also yields the softmax
  denominators (tail rows sitting below M_PAD are nulled by it).
  1. K^T = Wk^T x^T; Q^T = (0.125*Wq)^T x^T; V_aug per m-tile.
  2. per r-chunk, per head: S^T = K_h Q_h^T into grouped PSUM banks,
     P^T = exp(S^T) (ScalarE, one op per bank group, bf16 out).
  3. O^T[h] = V_aug_h^T P^T accumulated over m-tiles; row 0 is the
     denominator; recip (DVE) -> partition_broadcast (Pool) ->
     tensor_mul normalize into OnT (bf16).
  4. y = OnT^T Wo per 128-row output tile (one iteration later),
     ScalarE-cast to bf16 and DMA'd out; host sums the 4 partials per
     batch in fp32 and adds the bias.
"""

import numpy as np

_CORES = 8
_DIM = 512
_DH = 64
_H = 8
_INNER = _H * _DH

# fallback (flip if partition_broadcast fails on a new toolchain)
_PBCAST = True     # gpsimd partition_broadcast for recip replication


def _ceil_to(x, m):
    return ((x + m - 1) // m) * m


def build_bass(R_PAD, M_PAD):
    """Build the SPMD bass program for padded sizes R_PAD (queries) and
    M_PAD (kv columns). Returns the compiled Bacc object."""
    import concourse.bacc as bacc
    import concourse.mybir as mybir
    import concourse.tile as tile

    f32 = mybir.dt.float32
    f32r = mybir.dt.float32r
    bf16 = mybir.dt.bfloat16
    EXP = mybir.ActivationFunctionType.Exp

    assert R_PAD % 16 == 0 and M_PAD % 128 == 0 and R_PAD >= M_PAD
    NMT = M_PAD // 128          # kv m-tiles
    NRT = (R_PAD + 127) // 128  # query r-tiles for the final projection
    # r-chunks: full 512s plus a short tail (bf16 matmuls don't need >=256;
    # finer chunking was measured slower: per-chunk pipeline overhead
    # outweighs the shorter trailing chain)
    RC = []
    o = 0
    while o < R_PAD:
        RC.append((o, min(512, R_PAD - o)))
        o += 512
    MC = []
    o = 0
    while o < M_PAD:
        MC.append((o, min(512, M_PAD - o)))
        o += 512

    nc = bacc.Bacc("TRN2", target_bir_lowering=False, debug=False,
                   num_devices=_CORES)

    xT_d = nc.dram_tensor("xT", [512, R_PAD], bf16, kind="ExternalInput")
    wq_d = nc.dram_tensor("wq", [512, 128], bf16, kind="ExternalInput")
    wk_d = nc.dram_tensor("wk", [512, 128], bf16, kind="ExternalInput")
    wv_d = nc.dram_tensor("wv", [512, 128], bf16, kind="ExternalInput")
    kvc_d = nc.dram_tensor("kvc", [128, 2 * NMT], f32, kind="ExternalInput")
    wo_d = nc.dram_tensor("wo", [128, 512], bf16, kind="ExternalInput")
    y_d = nc.dram_tensor("y", [R_PAD, 512], bf16, kind="ExternalOutput")

    with tile.TileContext(nc) as tc:
        with (
            tc.tile_pool(name="consts", bufs=1) as consts,
            tc.tile_pool(name="pt", bufs=6) as ptpool,
            tc.tile_pool(name="rcp", bufs=3) as rpool,
            tc.tile_pool(name="ysb", bufs=3) as ypool,
            tc.tile_pool(name="psS0", bufs=1, space="PSUM") as psS0,
            tc.tile_pool(name="psS1", bufs=1, space="PSUM") as psS1,
            tc.tile_pool(name="po", bufs=2, space="PSUM") as po,
            tc.tile_pool(name="py", bufs=2, space="PSUM") as py,
        ):
            # ---- input DMAs: split across the two HWDGE queues (sync,
            # scalar) in need-order: the K projection consumes wk + xT
            # chunk-by-chunk first, so those transfers go to the front of
            # both queues (HBM read bandwidth is the serializer).
            wk = consts.tile([128, 4, 128], bf16, tag="wk")
            nc.sync.dma_start(
                out=wk, in_=wk_d.ap().rearrange("(a p) d -> p a d", p=128))
            wq = consts.tile([128, 4, 128], bf16, tag="wq")
            nc.scalar.dma_start(
                out=wq, in_=wq_d.ap().rearrange("(a p) d -> p a d", p=128))
            # xT streamed in half-chunks, alternating queues, so the K/Q
            # projection matmuls can start consuming chunk c while later
            # chunks are still in flight (the input phase is HBM-bound).
            xT = consts.tile([128, 4, R_PAD], bf16, tag="xT")
            HB = R_PAD // 2
            xeng = [nc.sync, nc.scalar]
            for c in range(4):
                for hf in range(2):
                    sl = slice(hf * HB, R_PAD if hf else HB)
                    xeng[(2 * c + hf) % 2].dma_start(
                        out=xT[:, c, sl],
                        in_=xT_d.ap()[c * 128:(c + 1) * 128, sl])
            wv = consts.tile([128, 4, 128], bf16, tag="wv")
            nc.sync.dma_start(
                out=wv, in_=wv_d.ap().rearrange("(a p) d -> p a d", p=128))
            kvc = consts.tile([128, NMT, 2], f32, tag="kvc")
            nc.scalar.dma_start(out=kvc, in_=kvc_d.ap())
            wo = consts.tile([128, 512], bf16, tag="wo")
            nc.scalar.dma_start(out=wo, in_=wo_d.ap())

            def psum_tile(i, name):
                # rotate [128,512] f32 psum scratch across the po/py pools
                pool = (po, py)[i % 2]
                return pool.tile([128, 512], f32, tag="big", name=name)



            # ---- phase 1: projections ------------------------------------
            # K first (S matmuls need all of KT), then Q chunk by chunk.
            # NOTE: gpsimd (Pool) cannot access PSUM, so every PSUM->SBUF
            # mover must be the scalar (activation) or vector (DVE) engine.
            KT = consts.tile([128, M_PAD], bf16, tag="KT")
            for i, (o, w) in enumerate(MC):
                ps = psum_tile(i, f"kps{i}")
                for c in range(4):
                    nc.tensor.matmul(ps[:, :w], wk[:, c, :], xT[:, c, o:o + w],
                                     start=(c == 0), stop=(c == 3))
                nc.vector.tensor_copy(KT[:, o:o + w], ps[:, :w])

            QT = consts.tile([128, R_PAD], bf16, tag="QT")
            for i, (o, w) in enumerate(RC):
                ps = psum_tile(i, f"qps{i}")
                for c in range(4):
                    nc.tensor.matmul(ps[:, :w], wq[:, c, :], xT[:, c, o:o + w],
                                     start=(c == 0), stop=(c == 3))
                nc.scalar.copy(QT[:, o:o + w], ps[:, :w])

            # V_aug layout per (mt, head): [kv1 | 63 zeros | V(64)] so the
            # attn@V output carries the softmax denominator at partition 0
            # and O at partitions 64:128. Rows are scaled by the kv
            # indicator to null tail rows sitting below M_PAD.
            vt = consts.tile([128, NMT, 2, 128], bf16, tag="vt")
            nc.gpsimd.memset(vt, 0.0)
            for mt in range(NMT):
                ps = psum_tile(mt, f"vps{mt}")
                sl = slice(mt * 128, (mt + 1) * 128)
                for c in range(4):
                    nc.tensor.matmul(ps[:, :128], xT[:, c, sl], wv[:, c, :],
                                     start=(c == 0), stop=(c == 3))
                for hh in range(2):
                    nc.gpsimd.tensor_copy(vt[:, mt, hh, 0:1],
                                          kvc[:, mt, hh:hh + 1])
                nc.vector.tensor_scalar_mul(
                    vt[:, mt, :, 64:128],
                    in0=ps[:, 0:128].rearrange("p (h c) -> p h c", h=2),
                    scalar1=kvc[:, mt, 0:1])

            # ---- phases 2+3, software-pipelined by one iteration ---------
            # iteration = (h, r-chunk); S matmuls of iter i overlap the
            # exp of iter i on ScalarE and the O/normalize of iter i-1.
            OnT = consts.tile([128, R_PAD], bf16, tag="OnT")

            # mt groupings: pairs of m-tiles share one 2-bank psum tile
            # and one exp instruction.
            GRPS = [(0, 1), (2, 3), (4,)]
            SPOOL = [psS0, psS1]

            def emit_S_pair(o, w, grps):
                """S^T matmuls + exp for both heads over the given m-tile
                groups. The two heads' matmuls are emitted adjacently: with
                64-partition contractions they occupy disjoint PE row
                groups and stream concurrently. Returns {h: [moving AP per
                m-tile, in order]}. Singleton groups and the tail chunk
                pack both heads into one psum tile / one exp."""
                pts = {0: [], 1: []}
                if w * NMT > 512:
                    for gi, g in grps:
                        ng = len(g)
                        if ng == 2:
                            sps = [SPOOL[h].tile([128, 2, 512], f32,
                                                 tag="sp",
                                                 name=f"sp{h}_{o}_{gi}")
                                   for h in (0, 1)]
                            for j, mt in enumerate(g):
                                msl = slice(mt * 128, (mt + 1) * 128)
                                for h in (0, 1):
                                    hs = slice(h * 64, (h + 1) * 64)
                                    nc.tensor.matmul(
                                        sps[h][:, j, :w], KT[hs, msl],
                                        QT[hs, o:o + w],
                                        start=True, stop=True)
                            for h in (0, 1):
                                pt = ptpool.tile([128, 2, 512], bf16,
                                                 tag="pt",
                                                 name=f"pt{h}_{o}_{gi}")
                                nc.scalar.activation(out=pt[:, :2, :w],
                                                     in_=sps[h][:, :2, :w],
                                                     func=EXP)
                                pts[h] += [pt[:, j, :w] for j in range(2)]
                        else:
                            # one m-tile: heads share the tile and the exp
                            mt = g[0]
                            msl = slice(mt * 128, (mt + 1) * 128)
                            sp = psS0.tile([128, 2, 512], f32, tag="sp",
                                           name=f"spm_{o}_{gi}")
                            for h in (0, 1):
                                hs = slice(h * 64, (h + 1) * 64)
                                nc.tensor.matmul(sp[:, h, :w], KT[hs, msl],
                                                 QT[hs, o:o + w],
                                                 start=True, stop=True)
                            pt = ptpool.tile([128, 2, 512], bf16, tag="pt",
                                             name=f"ptm_{o}_{gi}")
                            nc.scalar.activation(out=pt[:, :2, :w],
                                                 in_=sp[:, :2, :w], func=EXP)
                            for h in (0, 1):
                                pts[h].append(pt[:, h, :w])
                else:
                    # tail chunk: per head, all NMT m-tiles packed into one
                    # psum bank (head = slot); one exp for everything
                    if not grps or grps[0][0] != 0:
                        return pts
                    sp = psS0.tile([128, 2, 512], f32, tag="sp",
                                   name=f"spt_{o}")
                    for mt in range(NMT):
                        msl = slice(mt * 128, (mt + 1) * 128)
                        for h in (0, 1):
                            hs = slice(h * 64, (h + 1) * 64)
                            nc.tensor.matmul(sp[:, h, mt * w:(mt + 1) * w],
                                             KT[hs, msl], QT[hs, o:o + w],
                                             start=True, stop=True)
                    pt = ptpool.tile([128, 2, NMT, 16], bf16, tag="ptt",
                                     name=f"ptt_{o}")
                    nc.scalar.activation(
                        out=pt,
                        in_=sp[:, :2, :NMT * w].rearrange(
                            "p h (m c) -> p h m c", m=NMT),
                        func=EXP)
                    for h in (0, 1):
                        pts[h] += [pt[:, h, mt, :] for mt in range(NMT)]
                return pts

            def emit_O(h, o, w, mvs):
                """attn@V + normalize for one (h, r-chunk)."""
                ops = po.tile([128, 512], f32, tag="big", name=f"ops{h}_{o}")
                for mt, mv in enumerate(mvs):
                    nc.tensor.matmul(ops[:, :w], vt[:, mt, h, :], mv,
                                     start=(mt == 0), stop=(mt == NMT - 1))
                rcp = rpool.tile([1, 512], f32, tag="rcp", name=f"rcp{h}_{o}")
                nc.vector.reciprocal_approx_fast(rcp[:, :w], ops[0:1, :w])
                rep = rpool.tile([64, 512], f32, tag="rep", name=f"rep{h}_{o}")
                if _PBCAST:
                    nc.gpsimd.partition_broadcast(rep[:, :w], rcp[:, :w])
                else:
                    prep = py.tile([128, 512], f32, tag="big",
                                   name=f"prep{h}_{o}")
                    ones = getattr(emit_O, "_ones", None)
                    nc.tensor.matmul(prep[:64, :w],
                                     ones[0:1, :].bitcast(f32r),
                                     rcp[:, :w].bitcast(f32r),
                                     start=True, stop=True)
                    nc.vector.tensor_copy(rep[:, :w], prep[:64, :w])
                nc.vector.tensor_mul(OnT[h * 64:(h + 1) * 64, o:o + w],
                                     ops[64:128, :w], rep[:, :w])

            if not _PBCAST:
                ones = consts.tile([1, 64], f32, tag="ones")
                nc.vector.memset(ones, 1.0)
                emit_O._ones = ones

            ydone = 0
            cover = 0  # rows of OnT complete for both heads
            pend = None
            ycop = [nc.scalar, nc.vector]

            def emit_y(rt):
                tw = min(128, R_PAD - rt * 128)
                ps = py.tile([128, 512], f32, tag="big", name=f"yps{rt}")
                rsl = slice(rt * 128, rt * 128 + tw)
                nc.tensor.matmul(ps[:tw, :], OnT[:, rsl], wo,
                                 start=True, stop=True)
                ysb = ypool.tile([128, 512], bf16, tag="y", name=f"ysb{rt}")
                # always ScalarE: by the time y-tiles emit, the exps for
                # their region are done, and keeping these casts off the
                # DVE keeps the recip->mul normalize chains unclogged
                nc.scalar.copy(ysb[:tw, :], ps[:tw, :])
                nc.sync.dma_start(out=y_d.ap()[rsl, :], in_=ysb[:tw, :])
            # iteration = r-chunk; O+normalize of the previous chunk is
            # interleaved between this chunk's S groups so the PE never
            # waits on the exp of the group it just produced. Output
            # r-tiles of a chunk are projected one iteration after that
            # chunk's normalize (the recip->broadcast->mul chain is
            # ~2.3us; the PE must not arrive early).
            ready = []   # r-tiles whose chunk normalized >= 1 iter ago
            fresh = []   # r-tiles normalized during this iteration
            for i, (o, w) in enumerate(RC):
                gl = list(enumerate(GRPS))
                pts = emit_S_pair(o, w, gl[:1])
                if pend is not None:
                    emit_O(0, po_, pw, pend[0])
                pts2 = emit_S_pair(o, w, gl[1:2])
                for rt in ready:
                    emit_y(rt)
                ready = []
                if pend is not None:
                    emit_O(1, po_, pw, pend[1])
                    fresh = list(range(po_ // 128, (po_ + pw + 127) // 128))
                pts3 = emit_S_pair(o, w, gl[2:])
                for h in (0, 1):
                    pts[h] = pts[h] + pts2[h] + pts3[h]
                pend, po_, pw = pts, o, w
                ready, fresh = ready + fresh, []
            emit_O(0, po_, pw, pend[0])
            emit_O(1, po_, pw, pend[1])
            for rt in ready + list(range(po_ // 128, (po_ + pw + 127) // 128)):
                emit_y(rt)

    nc.compile()
    return nc


def _prep(x, mask_np, mask_bert, W_qkv, W_out):
    """Host-side gather/shard. Returns (in_maps, meta)."""
    import ml_dtypes
    bf16 = ml_dtypes.bfloat16

    B, N, DIM = x.shape
    assert (B, DIM) == (2, _DIM)
    x = np.ascontiguousarray(x, dtype=np.float32)
    W_qkv = np.ascontiguousarray(W_qkv, dtype=np.float32)
    W_out = np.ascontiguousarray(W_out, dtype=np.float32)

    kv_idx, tail_idx, Ms, tails = [], [], [], []
    for b in range(B):
        npb = mask_np[b].astype(bool)
        bb = mask_bert[b].astype(bool)
        kv = np.nonzero(npb & ~bb)[0]
        tl = np.nonzero(npb & bb)[0]
        kv_idx.append(kv)
        tail_idx.append(tl)
        Ms.append(len(kv))
        tails.append(len(tl))

    M_PAD = max(128, _ceil_to(max(Ms), 128))
    # rows are packed [kv | tail] with no gap: the tail rows that fall in
    # [M_b, M_PAD) act as key/value candidates but are nulled by the kvc
    # indicator (V rows scaled to 0, denominator column 0), so no zero gap
    # is needed and R_PAD shrinks to the real row count.
    R_PAD = max(128, _ceil_to(max(Ms[b] + tails[b] for b in range(B)), 16),
                M_PAD)

    NMT = M_PAD // 128
    xT_b, kvc_b, row_pos = [], [], []
    for b in range(B):
        xa = np.zeros((512, R_PAD), dtype=bf16)
        xa[:, :Ms[b]] = x[b][kv_idx[b]].T.astype(bf16)
        xa[:, Ms[b]:Ms[b] + tails[b]] = x[b][tail_idx[b]].T.astype(bf16)
        xT_b.append(xa)
        kvones = np.zeros(M_PAD, dtype=np.float32)
        kvones[:Ms[b]] = 1.0
        # [128, NMT, 2]: per m-tile kv indicator, duplicated per head slot
        kvt = np.repeat(kvones.reshape(NMT, 128).T[:, :, None], 2, axis=2)
        kvc_b.append(np.ascontiguousarray(kvt.reshape(128, 2 * NMT)))
        # output row p of the device result corresponds to token row_pos[p]
        pos = np.concatenate([kv_idx[b], tail_idx[b]])
        row_pos.append(pos)

    scale = np.float32(_DH ** -0.5)
    in_maps = []
    for c in range(_CORES):
        b, g = divmod(c, 4)
        qc = slice(128 * g, 128 * g + 128)
        kc = slice(_INNER + 128 * g, _INNER + 128 * g + 128)
        vc = slice(2 * _INNER + 128 * g, 2 * _INNER + 128 * g + 128)
        wq = np.ascontiguousarray((W_qkv[:, qc] * scale).astype(bf16))
        wk = np.ascontiguousarray(W_qkv[:, kc].astype(bf16))
        wv = np.ascontiguousarray(W_qkv[:, vc].astype(bf16))
        wo = np.ascontiguousarray(
            W_out[128 * g:128 * g + 128, :].astype(bf16))
        in_maps.append({"xT": xT_b[b], "wq": wq, "wk": wk, "wv": wv,
                        "wo": wo, "kvc": kvc_b[b]})

    meta = dict(M_PAD=M_PAD, R_PAD=R_PAD, Ms=Ms, tails=tails,
                kv_idx=kv_idx, tail_idx=tail_idx, row_pos=row_pos)
    return in_maps, meta


def _assemble(results, meta, x, mask_np, W_qkv, W_out, b_out):
    B, N, _ = x.shape
    out = np.empty((B, N, _DIM), dtype=np.float32)
    Wv_full = W_qkv[:, 2 * _INNER:].astype(np.float32)
    for b in range(B):
        # constant output for fully-masked rows: uniform attention = mean(V)
        meanv = (x[b].mean(axis=0, dtype=np.float32) @ Wv_full)
        yconst = meanv @ W_out.astype(np.float32) + b_out
        out[b, :, :] = yconst[None, :]
        Mb, tb = meta["Ms"][b], meta["tails"][b]
        if Mb == 0:
            # no unmasked kv columns: every row is fully masked -> uniform
            continue
        acc = None
        for g in range(4):
            yp = np.asarray(results[4 * b + g]["y"], dtype=np.float32)
            acc = yp.copy() if acc is None else acc + yp
        out[b, meta["row_pos"][b], :] = acc[:Mb + tb] + b_out
    return out


_CACHE = {}


def _get_bass(R_PAD, M_PAD):
    key = (R_PAD, M_PAD)
    if key not in _CACHE:
        _CACHE[key] = build_bass(R_PAD, M_PAD)
    return _CACHE[key]


def run_spmd(in_maps, meta, trace=False, tmpdir=None, trace_cores=None):
    from concourse.bass_utils import run_bass_kernel_spmd

    nc = _get_bass(meta["R_PAD"], meta["M_PAD"])
    return run_bass_kernel_spmd(
        nc, in_maps, core_ids=list(range(_CORES)), trace=trace, tmpdir=tmpdir,
        trace_cores=trace_cores)


def kernel(x, mask_np, mask_bert, W_qkv, W_out, b_out):
    x = np.asarray(x)
    mask_np = np.asarray(mask_np)
    mask_bert = np.asarray(mask_bert)
    W_qkv = np.asarray(W_qkv, dtype=np.float32)
    W_out = np.asarray(W_out, dtype=np.float32)
    b_out = np.asarray(b_out, dtype=np.float32)

    in_maps, meta = _prep(x, mask_np, mask_bert, W_qkv, W_out)
    res = run_spmd(in_maps, meta)
    return _assemble(res.results, meta, x, mask_np, W_qkv, W_out, b_out)
